# revision 12
# baseline (speedup 1.0000x reference)
"""Self-contained Trainium2 kernel for the dense transformer block problem.

kernel(**inputs) takes the FULL inputs (as produced by the reference
setup_inputs), shards them across 8 NeuronCores (2 cores per batch element,
causal-balanced parity split of query blocks), runs a Bass/Tile SPMD kernel,
and reassembles the full [B, T, C] output.

Sharding: 2 cores per batch element (B=4). Within a pair, query blocks of 128
tokens are split by parity (core parity p owns global blocks {2j+p}), which
balances causal attention work. Each core computes K/V for the full sequence
of its batch element (redundant within the pair) so there are no collectives.

v2: bf16 compute pipeline (weights, activations, attention) with f32 residual
stream; attn/x2 kept in SBUF (no DRAM roundtrips); single-pass MLP.
"""
import sys
sys.path.insert(0, '/opt/trn_rl_repo')
import numpy as np
from contextlib import ExitStack

import concourse.bacc as bacc
import concourse.tile as tile
import concourse.mybir as mybir
from concourse.masks import make_identity

F32 = mybir.dt.float32
F32R = mybir.dt.float32r
BF16 = mybir.dt.bfloat16
AF = mybir.ActivationFunctionType
ALU = mybir.AluOpType

B, T, C, H, DH = 4, 2048, 1024, 16, 64
N_CORES = 8
TOK = 1024          # own tokens per core
NB = TOK // 128     # 8 own query blocks
KB = T // 128       # 16 key blocks
CCH = C // 128      # 8 channel chunks
FF = 4 * C          # 4096
FCH = FF // 128     # 32 ff chunks
EPS = 1e-5

IN_NAMES = ["xfull", "xown", "qpos", "Wq", "Wk", "Wv", "Wp", "bp",
            "W1", "b1", "W2", "b2", "qbias", "kbias", "vbias"]

STAGE_MARKS = []   # [(instruction_number, stage_name)] filled during build


def _mark(nc, name):
    try:
        n = int(nc.get_next_instruction_name().rsplit("-", 1)[1])
        STAGE_MARKS.append((n, name))
    except Exception:
        pass


def build(nc, reps=1):
    """Trace the SPMD program into nc (a bacc.Bacc). Call nc.compile() after.

    Weight inputs arrive pre-folded on the host (bf16):
      Wq/Wk/Wv = diag(g1) @ W;  qbias/kbias/vbias = be1 @ W
      W1 = diag(g2) @ W1;  b1 = b1 + be2 @ W1
      Wp, W2 plain.  g/be tensors are consumed host-side only.
    """
    def din(name, shape, dt=F32):
        return nc.dram_tensor(name, shape, dt, kind="ExternalInput")

    xfull_d = din("xfull", [T, C])
    xown_d = din("xown", [TOK, C])
    qpos_d = din("qpos", [NB, 128])
    Wq_d = din("Wq", [C, C], BF16); Wk_d = din("Wk", [C, C], BF16)
    Wv_d = din("Wv", [C, C], BF16); Wp_d = din("Wp", [C, C], BF16)
    bp_d = din("bp", [1, C]); W1_d = din("W1", [C, FF], BF16); b1_d = din("b1", [1, FF])
    W2_d = din("W2", [FF, C], BF16); b2_d = din("b2", [1, C])
    qb_d = din("qbias", [NB, 128])   # be1 @ Wq, laid out [pair, dh-stacked 128]
    kb_d = din("kbias", [NB, 128])   # be1 @ Wk
    vb_d = din("vbias", [1, C])      # be1 @ Wv
    out_d = nc.dram_tensor("out", [TOK, C], F32, kind="ExternalOutput")

    Wqv = Wq_d.ap().rearrange("(o p) m -> o p m", p=128)
    Wkv = Wk_d.ap().rearrange("(o p) m -> o p m", p=128)
    Wvv = Wv_d.ap().rearrange("(o p) m -> o p m", p=128)
    Wpv = Wp_d.ap().rearrange("(o p) m -> o p m", p=128)
    W1v = W1_d.ap().rearrange("(o p) m -> o p m", p=128)
    W2v = W2_d.ap().rearrange("(o p) m -> p o m", p=128)  # [128, 32, 1024]
    xf = xfull_d.ap()
    xo = xown_d.ap()

    for _rep in range(reps):
        _build_one(nc, locals())
    return IN_NAMES


def _build_one(nc, env):
    (xfull_d, xown_d, qpos_d, Wq_d, Wk_d, Wv_d, Wp_d, bp_d, W1_d, b1_d, W2_d,
     b2_d, qb_d, kb_d, vb_d, out_d, Wqv, Wkv, Wvv, Wpv, W1v, W2v,
     xf, xo) = (
        env[k] for k in ["xfull_d", "xown_d", "qpos_d", "Wq_d", "Wk_d", "Wv_d",
                         "Wp_d", "bp_d", "W1_d", "b1_d", "W2_d", "b2_d", "qb_d",
                         "kb_d", "vb_d", "out_d", "Wqv", "Wkv",
                         "Wvv", "Wpv", "W1v", "W2v", "xf", "xo"])
    import concourse.tile as tile
    from contextlib import ExitStack
    with tile.TileContext(nc) as tc, ExitStack() as top:
        const = top.enter_context(tc.tile_pool(name="const", bufs=1))
        ident = const.tile([128, 128], BF16)
        make_identity(nc, ident[:])
        eps_t = const.tile([128, 1], F32)
        nc.vector.memset(eps_t[:], EPS)

        def ln_stats(nc, pool, x_ap):
            n = x_ap.shape[-1] // 512
            xg = x_ap.rearrange("p (n f) -> p n f", f=512)
            stats = pool.tile([128, n, 6], F32, tag="ln_stats")
            mv = pool.tile([128, 2], F32, tag="ln_mv")
            for i in range(n):
                nc.vector.bn_stats(stats[:, i], xg[:, i])
            nc.vector.bn_aggr(mv[:], stats[:])
            rstd = pool.tile([128, 1], F32, tag="ln_rstd")
            nc.scalar.activation(rstd[:], mv[:, 1:2], AF.Sqrt, bias=eps_t[:])
            nc.vector.reciprocal(rstd[:], rstd[:])
            return mv[:, 0:1], rstd

        def ln_apply(nc, pool, out_ap, x_ap, mean, rstd):
            # out = (x - mu) * rstd on ACT: Identity(x * rstd + (-mu * rstd))
            nmr = pool.tile([128, 1], F32, tag="ln_nmr")
            nc.vector.tensor_scalar(nmr[:], mean, rstd[:], -1.0,
                                    op0=ALU.mult, op1=ALU.mult)
            nc.scalar.activation(out_ap, x_ap, AF.Identity,
                                 bias=nmr[:], scale=rstd[:])

        # persistent SBUF: x2 (residual after attn) and ff2 (MLP out), token-major
        x2p = top.enter_context(tc.tile_pool(name="x2sb", bufs=1))
        x2_sb = x2p.tile([128, NB, C], F32)

        _mark(nc, 'A_ln1full')
        # ============ Stage A: LN1 over full T -> hT [128, CCH, T] bf16 ============
        es_h = ExitStack()
        hp = es_h.enter_context(tc.tile_pool(name="hT", bufs=1, side="right"))
        hT = hp.tile([128, CCH, T], BF16)
        with tc.tile_pool(name="stA", bufs=3) as stA, \
             tc.tile_pool(name="stA_ps", bufs=3, space="PSUM") as psA:
            for tb in range(T // 128):
                x_t = stA.tile([128, C], F32, tag="x_t")
                nc.sync.dma_start(x_t[:], xf[tb * 128:(tb + 1) * 128, :])
                mean, rstd = ln_stats(nc, stA, x_t[:])
                hrow = stA.tile([128, C], BF16, tag="hrow")
                ln_apply(nc, stA, hrow[:], x_t[:], mean, rstd)
                for cc in range(CCH):
                    pt = psA.tile([128, 128], BF16, tag="psA_t")
                    nc.tensor.transpose(pt[:], hrow[:, cc * 128:(cc + 1) * 128], ident[:])
                    eng = nc.scalar.copy if cc % 2 == 0 else nc.vector.tensor_copy
                    eng(hT[:, cc, tb * 128:(tb + 1) * 128], pt[:])

        _mark(nc, 'B1_V')
        # ============ Stage B1: V (token-major, bf16, ones-augmented) ============
        es_qkv = ExitStack()
        vp = es_qkv.enter_context(tc.tile_pool(name="Vp", bufs=1))
        V_sb = vp.tile([128, KB, H, 65], BF16)
        ones_f = vp.tile([128, 1], F32)
        nc.vector.memset(ones_f[:], 1.0)
        ones_r = vp.tile([128, 1], BF16)
        nc.vector.tensor_copy(ones_r[:], ones_f[:])
        nc.vector.tensor_copy(V_sb[:, :, :, 64:65],
                              ones_r[:, 0:1, None, None].to_broadcast([128, KB, H, 1]))
        with tc.tile_pool(name="stB1a", bufs=2) as stB1a, \
             tc.tile_pool(name="stB1c", bufs=1) as stB1c, \
             tc.tile_pool(name="stB1_ps", bufs=2, space="PSUM") as psB1:
            vb_b = stB1c.tile([128, C], F32)
            nc.sync.dma_start(vb_b[:], vb_d.ap().to_broadcast([128, C]))
            for grp in range(2):
                wv_g = stB1a.tile([128, CCH, 512], BF16, tag="wv_g")
                nc.sync.dma_start(wv_g[:], Wvv.transpose([1, 0, 2])[:, :, grp * 512:(grp + 1) * 512])
                for tb in range(KB):
                    pv = psB1.tile([128, 512], F32, tag="pv")
                    for cc in range(CCH):
                        nc.tensor.matmul(pv[:], hT[:, cc, tb * 128:(tb + 1) * 128],
                                         wv_g[:, cc], start=(cc == 0), stop=(cc == CCH - 1))
                    nc.vector.tensor_tensor(
                        V_sb[:, tb, grp * 8:(grp + 1) * 8, 0:64],
                        pv[:].rearrange("p (h d) -> p h d", d=64),
                        vb_b[:, grp * 512:(grp + 1) * 512].rearrange("p (h d) -> p h d", d=64),
                        ALU.add)

        _mark(nc, 'B2_K')
        # ============ Stage B2: KT [128(dh pair-stacked), pair, T] bf16 ============
        ktp = es_qkv.enter_context(tc.tile_pool(name="KTp", bufs=1))
        KT = ktp.tile([128, CCH, T], BF16)
        with tc.tile_pool(name="stB2", bufs=2) as stB2, \
             tc.tile_pool(name="stB2c", bufs=1) as stB2c, \
             tc.tile_pool(name="stB2_ps", bufs=3, space="PSUM") as psB2:
            kb_sb = stB2c.tile([128, NB], F32)
            nc.sync.dma_start(kb_sb[:], kb_d.ap().rearrange("o p -> p o"))
            for pair in range(CCH):
                wk_p = stB2.tile([128, CCH, 128], BF16, tag="wk_p")
                nc.sync.dma_start(wk_p[:], Wkv.transpose([1, 0, 2])[:, :, pair * 128:(pair + 1) * 128])
                for nt in range(T // 512):
                    pk = psB2.tile([128, 512], F32, tag="pk")
                    for cc in range(CCH):
                        nc.tensor.matmul(pk[:], wk_p[:, cc],
                                         hT[:, cc, nt * 512:(nt + 1) * 512],
                                         start=(cc == 0), stop=(cc == CCH - 1))
                    nc.vector.tensor_scalar(KT[:, pair, nt * 512:(nt + 1) * 512], pk[:],
                                            kb_sb[:, pair:pair + 1], None, op0=ALU.add)

        _mark(nc, 'A2_ln1own')
        # ============ Stage A': LN1 of own rows -> hTown; then B3: QT ============
        es_h.close()  # free hT
        es_ho = ExitStack()
        hop = es_ho.enter_context(tc.tile_pool(name="hTown", bufs=1, side="right"))
        hTown = hop.tile([128, CCH, TOK], BF16)
        with tc.tile_pool(name="stA2", bufs=3) as stA2, \
             tc.tile_pool(name="stA2_ps", bufs=3, space="PSUM") as psA2:
            for tb in range(NB):
                x_t = stA2.tile([128, C], F32, tag="x_t2")
                nc.sync.dma_start(x_t[:], xo[tb * 128:(tb + 1) * 128, :])
                mean, rstd = ln_stats(nc, stA2, x_t[:])
                hrow = stA2.tile([128, C], BF16, tag="hrow2")
                ln_apply(nc, stA2, hrow[:], x_t[:], mean, rstd)
                for cc in range(CCH):
                    pt = psA2.tile([128, 128], BF16, tag="psA2_t")
                    nc.tensor.transpose(pt[:], hrow[:, cc * 128:(cc + 1) * 128], ident[:])
                    eng = nc.scalar.copy if cc % 2 == 0 else nc.vector.tensor_copy
                    eng(hTown[:, cc, tb * 128:(tb + 1) * 128], pt[:])

        _mark(nc, 'B3_Q')
        qtp = es_qkv.enter_context(tc.tile_pool(name="QTp", bufs=1))
        QT = qtp.tile([128, CCH, TOK], BF16)
        with tc.tile_pool(name="stB3", bufs=2) as stB3, \
             tc.tile_pool(name="stB3c", bufs=1) as stB3c, \
             tc.tile_pool(name="stB3_ps", bufs=3, space="PSUM") as psB3:
            qb_sb = stB3c.tile([128, NB], F32)
            nc.sync.dma_start(qb_sb[:], qb_d.ap().rearrange("o p -> p o"))
            for pair in range(CCH):
                wq_p = stB3.tile([128, CCH, 128], BF16, tag="wq_p")
                nc.sync.dma_start(wq_p[:], Wqv.transpose([1, 0, 2])[:, :, pair * 128:(pair + 1) * 128])
                for nt in range(TOK // 512):
                    pq = psB3.tile([128, 512], F32, tag="pq")
                    for cc in range(CCH):
                        nc.tensor.matmul(pq[:], wq_p[:, cc],
                                         hTown[:, cc, nt * 512:(nt + 1) * 512],
                                         start=(cc == 0), stop=(cc == CCH - 1))
                    nc.vector.tensor_scalar(QT[:, pair, nt * 512:(nt + 1) * 512], pq[:],
                                            qb_sb[:, pair:pair + 1], None, op0=ALU.add)
        es_ho.close()  # free hTown

        # attn output, dh-major in SBUF: [128(pair-part), CCH, TOK] bf16
        es_attn = ExitStack()
        atp = es_attn.enter_context(tc.tile_pool(name="attnT", bufs=1, side="right"))
        attnT_sb = atp.tile([128, CCH, TOK], BF16)

        # ---------- mask constants (scoped to attention) ----------
        es_mask = ExitStack()
        maskp = es_mask.enter_context(tc.tile_pool(name="maskp", bufs=1, side="right"))
        kp_i = maskp.tile([128, KB], mybir.dt.int32)
        nc.gpsimd.iota(kp_i[:], pattern=[[128, KB]], base=0, channel_multiplier=1)
        kp_f = maskp.tile([128, KB], F32)
        nc.vector.tensor_copy(kp_f[:], kp_i[:])
        qb = maskp.tile([128, NB, 128], F32)
        for j in range(NB):
            nc.sync.dma_start(qb[:, j], qpos_d.ap()[j:j + 1, :].to_broadcast([128, 128]))
        biasm = maskp.tile([128, NB, 2, 128], BF16)
        for j in range(NB):
            for t in range(2):
                # m01[p_key, f_q] = (qpos_j[f] >= keypos(k=2j+t)[p])
                nc.vector.tensor_scalar(
                    biasm[:, j, t], qb[:, j], kp_f[:, 2 * j + t:2 * j + t + 1], None,
                    op0=ALU.is_ge)

        _mark(nc, 'C_attn')
        # ============ Stage C: attention ============
        with tc.tile_pool(name="stC", bufs=3) as stC, \
             tc.tile_pool(name="stC_att_ps", bufs=2, space="PSUM") as psCa, \
             tc.tile_pool(name="stC_s_ps", bufs=2, space="PSUM") as psCs, \
             tc.tile_pool(name="stC_t_ps", bufs=2, space="PSUM") as psCt:
            for h in range(H):
                pair, off = h // 2, 64 * (h % 2)
                ps_att = psCa.tile([128, TOK], F32, tag="ps_att")
                for k in range(KB):
                    jmin = k // 2
                    q0 = jmin * 128
                    nq = TOK - q0
                    weiT = stC.tile([128, TOK], BF16, tag="weiT")
                    qa = 0
                    while qa < nq:  # one 1-bank psum tile + one exp per 512 cols
                        qn = min(512, nq - qa)
                        ps_s = psCs.tile([128, 512], F32, tag="ps_s")
                        nc.tensor.matmul(
                            ps_s[:, 0:qn],
                            KT[off:off + 64, pair, k * 128:(k + 1) * 128],
                            QT[off:off + 64, pair, q0 + qa:q0 + qa + qn],
                            start=True, stop=True)
                        nc.scalar.activation(weiT[:, qa:qa + qn], ps_s[:, 0:qn],
                                             AF.Exp, scale=0.125)
                        qa += qn
                    nc.vector.tensor_tensor(weiT[:, 0:128], weiT[:, 0:128],
                                            biasm[:, jmin, k - 2 * jmin], ALU.mult)
                    # AV: one matmul per 512-col PSUM bank (start=True must
                    # clear a whole bank, so groups are bank-aligned)
                    if k <= 7:  # bank 0: q cols [q0, 512)
                        nc.tensor.matmul(
                            ps_att[0:65, q0:512],
                            V_sb[:, k, h, :],
                            weiT[:, 0:512 - q0],
                            start=(k == 0), stop=(k == 7))
                    b1lo = max(512, q0)  # bank 1: q cols [b1lo, 1024)
                    nc.tensor.matmul(
                        ps_att[0:65, b1lo:TOK],
                        V_sb[:, k, h, :],
                        weiT[:, b1lo - q0:TOK - q0],
                        start=(k == 0), stop=(k == KB - 1))
                # normalize + transpose into attnT_sb
                for j in range(NB):
                    sb_at = stC.tile([128, 128], BF16, tag="sb_at")
                    nc.vector.tensor_copy(sb_at[0:65, :], ps_att[0:65, j * 128:(j + 1) * 128])
                    pt1 = psCt.tile([128, 128], BF16, tag="ptn")
                    nc.tensor.transpose(pt1[:], sb_at[:], ident[:])
                    recip = stC.tile([128, 1], F32, tag="recip")
                    nc.vector.reciprocal(recip[:], pt1[:, 64:65])
                    attn_j = stC.tile([128, 64], BF16, tag="attn_j")
                    nc.vector.tensor_scalar_mul(attn_j[:], pt1[:, 0:64], recip[:])
                    pt2 = psCt.tile([128, 128], BF16, tag="ptn")
                    nc.tensor.transpose(pt2[off:off + 64, :], attn_j[:], ident[:])
                    nc.vector.tensor_copy(
                        attnT_sb[off:off + 64, pair, j * 128:(j + 1) * 128],
                        pt2[off:off + 64, :])
        es_mask.close()  # free mask constants
        es_qkv.close()   # free V, KT, QT

        _mark(nc, 'D_wp_ln2')
        # ============ Stage D: Wp proj + residual + LN2 ============
        es_h2 = ExitStack()
        h2p = es_h2.enter_context(tc.tile_pool(name="h2T", bufs=1))
        h2T = h2p.tile([128, CCH, TOK], BF16)
        with tc.tile_pool(name="stD", bufs=2) as stD, \
             tc.tile_pool(name="stD_c", bufs=1) as stDc, \
             tc.tile_pool(name="stD_ps", bufs=2, space="PSUM") as psD, \
             tc.tile_pool(name="stD_t_ps", bufs=2, space="PSUM") as psDt:
            bpb = stDc.tile([128, C], F32)
            nc.sync.dma_start(bpb[:], bp_d.ap().to_broadcast([128, C]))
            for nt in range(TOK // 512):
                pT_sb = stD.tile([128, CCH, 512], BF16, tag="pT_sb")
                for co in range(CCH):
                    pp = psD.tile([128, 512], F32, tag="pp")
                    wp_c = stD.tile([128, CCH, 128], BF16, tag="wp_c")
                    nc.sync.dma_start(wp_c[:], Wpv.transpose([1, 0, 2])[:, :, co * 128:(co + 1) * 128])
                    for cc in range(CCH):
                        nc.tensor.matmul(pp[:], wp_c[:, cc],
                                         attnT_sb[:, cc, nt * 512:(nt + 1) * 512],
                                         start=(cc == 0), stop=(cc == CCH - 1))
                    nc.scalar.copy(pT_sb[:, co], pp[:])
                for sub in range(4):
                    tb = nt * 4 + sub
                    xo_t = stD.tile([128, C], F32, tag="xo_t")
                    nc.sync.dma_start(xo_t[:], xo[tb * 128:(tb + 1) * 128, :])
                    x2_t = x2_sb[:, tb]
                    for co in range(CCH):
                        ptd = psDt.tile([128, 128], BF16, tag="ptd")
                        nc.tensor.transpose(ptd[:], pT_sb[:, co, sub * 128:(sub + 1) * 128],
                                            ident[:])
                        nc.vector.tensor_tensor(x2_t[:, co * 128:(co + 1) * 128], ptd[:],
                                                xo_t[:, co * 128:(co + 1) * 128], ALU.add)
                    nc.vector.tensor_tensor(x2_t[:], x2_t[:], bpb[:], ALU.add)
                    # LN2
                    mean, rstd = ln_stats(nc, stD, x2_t[:])
                    h2row = stD.tile([128, C], BF16, tag="h2row")
                    ln_apply(nc, stD, h2row[:], x2_t[:], mean, rstd)
                    for cc in range(CCH):
                        pt = psDt.tile([128, 128], BF16, tag="ptd2")
                        nc.tensor.transpose(pt[:], h2row[:, cc * 128:(cc + 1) * 128], ident[:])
                        eng = nc.scalar.copy if cc % 2 == 0 else nc.vector.tensor_copy
                        eng(h2T[:, cc, tb * 128:(tb + 1) * 128], pt[:])

        es_attn.close()  # free attnT
        _mark(nc, 'X_b2fold')
        # fold b2 into x2 now that LN2 has consumed x2 (out = x2 + b2 + ff)
        with tc.tile_pool(name="stX2b", bufs=1) as stX2b:
            b2b = stX2b.tile([128, C], F32)
            nc.sync.dma_start(b2b[:], b2_d.ap().to_broadcast([128, C]))
            for tb in range(NB):
                nc.vector.tensor_tensor(x2_sb[:, tb], x2_sb[:, tb], b2b[:], ALU.add)

        _mark(nc, 'E1_w1')
        # ============ Stage E: single-pass MLP (ff1T bf16 fits SBUF) ============
        es_ff = ExitStack()
        ffp = es_ff.enter_context(tc.tile_pool(name="ff1T", bufs=1))
        ff1T = ffp.tile([128, FCH, TOK], BF16)
        with tc.tile_pool(name="stE_c", bufs=1) as stEc:
            b1p = stEc.tile([128, FCH], F32)
            nc.sync.dma_start(b1p[:], b1_d.ap().rearrange("x (o p) -> p (x o)", p=128))
            with tc.tile_pool(name="stE1", bufs=2) as stE1, \
                 tc.tile_pool(name="stE1_ps", bufs=2, space="PSUM") as psE1:
                for fog in range(8):
                    w1g = stE1.tile([128, CCH, 512], BF16, tag="w1g")
                    nc.sync.dma_start(
                        w1g[:], W1v.transpose([1, 0, 2])[:, :, fog * 512:(fog + 1) * 512])
                    for f4 in range(4):
                        fo = fog * 4 + f4
                        for nt in range(TOK // 512):
                            pf = psE1.tile([128, 512], F32, tag="pf")
                            for cc in range(CCH):
                                nc.tensor.matmul(
                                    pf[:], w1g[:, cc, f4 * 128:(f4 + 1) * 128],
                                    h2T[:, cc, nt * 512:(nt + 1) * 512],
                                    start=(cc == 0), stop=(cc == CCH - 1))
                            nc.scalar.activation(
                                ff1T[:, fo, nt * 512:(nt + 1) * 512], pf[:], AF.Relu,
                                bias=b1p[:, fo:fo + 1])
            _mark(nc, 'E2F_w2out')
            # ========== Stage E2+F: W2, transpose, +x2(+b2), output ==========
            with tc.tile_pool(name="stE2", bufs=2) as stE2, \
                 tc.tile_pool(name="stE2_ps", bufs=2, space="PSUM") as psE2, \
                 tc.tile_pool(name="stF_ps", bufs=2, space="PSUM") as psF:
                for co in range(CCH):
                    w2c = stE2.tile([128, FCH, 128], BF16, tag="w2c")
                    nc.sync.dma_start(
                        w2c[:], W2v[:, :, co * 128:(co + 1) * 128])
                    for nt in range(TOK // 512):
                        p2 = psE2.tile([128, 512], F32, tag="p2")
                        for fo in range(FCH):
                            nc.tensor.matmul(p2[:], w2c[:, fo],
                                             ff1T[:, fo, nt * 512:(nt + 1) * 512],
                                             start=(fo == 0), stop=(fo == FCH - 1))
                        f2 = stE2.tile([128, 512], BF16, tag="f2")
                        nc.scalar.copy(f2[:], p2[:])
                        for sub in range(4):
                            tb = nt * 4 + sub
                            ptf = psF.tile([128, 128], BF16, tag="ptf")
                            nc.tensor.transpose(ptf[:], f2[:, sub * 128:(sub + 1) * 128],
                                                ident[:])
                            nc.vector.tensor_tensor(
                                x2_sb[:, tb, co * 128:(co + 1) * 128],
                                x2_sb[:, tb, co * 128:(co + 1) * 128],
                                ptf[:], ALU.add)
            for tb in range(NB):
                nc.sync.dma_start(out_d.ap()[tb * 128:(tb + 1) * 128, :], x2_sb[:, tb])
        es_ff.close()
        es_h2.close()  # free h2T (kept open past E1 for pool stack order)


def make_nc():
    nc = bacc.Bacc("TRN2", target_bir_lowering=False, debug=False,
                   num_devices=N_CORES)
    build(nc)
    nc.compile()
    return nc


def shard_inputs(inputs):
    """Full inputs dict -> list of 8 per-core in_maps.

    Folds LN1 gain/bias into Wq/Wk/Wv (weights scaled by g1 per input channel,
    be1 contribution becomes an additive bias on q/k/v) and LN2's into W1/b1.
    Weight matrices are cast to bf16 for the device.
    """
    import ml_dtypes
    bf16 = ml_dtypes.bfloat16
    x = np.asarray(inputs["x"], np.float32)
    assert x.shape == (B, T, C)
    f64 = np.float64
    Wq = np.asarray(inputs["Wq"], f64); Wk = np.asarray(inputs["Wk"], f64)
    Wv = np.asarray(inputs["Wv"], f64); Wp = np.asarray(inputs["Wp"], f64)
    W1 = np.asarray(inputs["W1"], f64); W2 = np.asarray(inputs["W2"], f64)
    g1 = np.asarray(inputs["g1"], f64); be1 = np.asarray(inputs["be1"], f64)
    g2 = np.asarray(inputs["g2"], f64); be2 = np.asarray(inputs["be2"], f64)
    b1 = np.asarray(inputs["b1"], f64)
    shared = {
        "Wq": (g1[:, None] * Wq).astype(bf16),
        "Wk": (g1[:, None] * Wk).astype(bf16),
        "Wv": (g1[:, None] * Wv).astype(bf16),
        "Wp": Wp.astype(bf16), "W2": W2.astype(bf16),
        "W1": (g2[:, None] * W1).astype(bf16),
        "qbias": (be1 @ Wq).astype(np.float32).reshape(NB, 128),
        "kbias": (be1 @ Wk).astype(np.float32).reshape(NB, 128),
        "vbias": (be1 @ Wv).astype(np.float32).reshape(1, C),
        "b1": (b1 + be2 @ W1).astype(np.float32).reshape(1, FF),
        "bp": np.asarray(inputs["bp"], np.float32).reshape(1, C),
        "b2": np.asarray(inputs["b2"], np.float32).reshape(1, C),
    }
    in_maps = []
    for c in range(N_CORES):
        b, par = c // 2, c % 2
        gblocks = [2 * j + par for j in range(NB)]
        rows = np.concatenate([x[b, g * 128:(g + 1) * 128, :] for g in gblocks], 0)
        qpos = np.stack([np.arange(g * 128, (g + 1) * 128, dtype=np.float32)
                         for g in gblocks], 0)
        m = {"xfull": np.ascontiguousarray(x[b]),
             "xown": np.ascontiguousarray(rows), "qpos": qpos}
        m.update(shared)
        in_maps.append(m)
    return in_maps


def unshard_outputs(results):
    """list of per-core {'out': [TOK, C]} -> [B, T, C]"""
    out = np.zeros((B, T, C), np.float32)
    for c in range(N_CORES):
        b, par = c // 2, c % 2
        r = np.asarray(results[c]["out"])
        for j in range(NB):
            g = 2 * j + par
            out[b, g * 128:(g + 1) * 128, :] = r[j * 128:(j + 1) * 128, :]
    return out


_NC_CACHE = {}

def _get_nc():
    if "nc" not in _NC_CACHE:
        nc = bacc.Bacc("TRN2", target_bir_lowering=False, debug=False,
                       num_devices=N_CORES)
        build(nc, reps=1)
        nc.compile()
        _NC_CACHE["nc"] = nc
    return _NC_CACHE["nc"]


def kernel(**inputs):
    from concourse.bass_utils import run_bass_kernel_spmd
    nc = _get_nc()
    in_maps = shard_inputs(inputs)
    res = run_bass_kernel_spmd(nc, in_maps, core_ids=list(range(N_CORES)))
    return unshard_outputs(res.results)


# revision 38
# speedup vs baseline: 53.2893x; 53.2893x over previous
"""Self-contained Trainium2 kernel for the dense transformer block problem.

kernel(**inputs) takes the FULL inputs (as produced by the reference
setup_inputs), shards them across 8 NeuronCores (2 cores per batch element,
causal-balanced parity split of query blocks), runs a Bass/Tile SPMD kernel,
and reassembles the full [B, T, C] output.

Sharding: 2 cores per batch element (B=4). Within a pair, query blocks of 128
tokens are split by parity (core parity p owns global blocks {2j+p}), which
balances causal attention work. Each core computes K/V for the full sequence
of its batch element (redundant within the pair) so there are no collectives.

v4: bf16 pipeline (f32 residual via xbp); XBAR DMA transposes for activation
layout changes; Wp/W2 outputs computed token-major (activation-stationary
matmuls) so no output transposes; batched-phase LayerNorm; A'/B3 fully
independent of B1/B2 (no SBUF reuse hazard); single-pass MLP.
"""
import sys
sys.path.insert(0, '/opt/trn_rl_repo')
import numpy as np
from contextlib import ExitStack

import concourse.bacc as bacc
import concourse.tile as tile
import concourse.mybir as mybir

F32 = mybir.dt.float32
BF16 = mybir.dt.bfloat16
AF = mybir.ActivationFunctionType
ALU = mybir.AluOpType

B, T, C, H, DH = 4, 2048, 1024, 16, 64
N_CORES = 8
TOK = 1024          # own tokens per core
NB = TOK // 128     # 8 own query blocks
KB = T // 128       # 16 key blocks
CCH = C // 128      # 8 channel chunks
FF = 4 * C          # 4096
FCH = FF // 128     # 32 ff chunks
EPS = 1e-5

IN_NAMES = ["xfbf", "xobf", "xbp", "qpos", "Wq", "Wk", "Wv", "Wp",
            "W1", "b1", "W2", "b2", "qbias", "kbias", "vbias"]

STAGE_MARKS = []   # [(instruction_number, stage_name)] filled during build


def _mark(nc, name):
    try:
        n = int(nc.get_next_instruction_name().rsplit("-", 1)[1])
        STAGE_MARKS.append((n, name))
    except Exception:
        pass


def build(nc, reps=1):
    """Trace the SPMD program into nc (a bacc.Bacc). Call nc.compile() after.

    Weight inputs arrive pre-folded on the host (bf16):
      Wq/Wk/Wv = diag(g1) @ W;  qbias/kbias/vbias = be1 @ W
      W1 = diag(g2) @ W1;  b1 = b1 + be2 @ W1;  xbp = xown + bp (f32)
      xfbf/xobf are bf16 copies of x (LN inputs only).
    """
    def din(name, shape, dt=F32):
        return nc.dram_tensor(name, shape, dt, kind="ExternalInput")

    xf_d = din("xfbf", [T, C], BF16)
    xo_d = din("xobf", [TOK, C], BF16)
    xbp_d = din("xbp", [TOK, C])
    qpos_d = din("qpos", [NB, 128])
    Wq_d = din("Wq", [C, C], BF16); Wk_d = din("Wk", [C, C], BF16)
    Wv_d = din("Wv", [C, C], BF16); Wp_d = din("Wp", [C, C], BF16)
    W1_d = din("W1", [C, FF], BF16); b1_d = din("b1", [1, FF])
    W2_d = din("W2", [FF, C], BF16); b2_d = din("b2", [1, C])
    qb_d = din("qbias", [NB, 128])   # be1 @ Wq, laid out [pair, dh-stacked 128]
    kb_d = din("kbias", [NB, 128])   # be1 @ Wk
    vb_d = din("vbias", [1, C])      # be1 @ Wv
    out_d = nc.dram_tensor("out", [TOK, C], F32, kind="ExternalOutput")

    Wqv = Wq_d.ap().rearrange("(o p) m -> o p m", p=128)
    Wkv = Wk_d.ap().rearrange("(o p) m -> o p m", p=128)
    Wvv = Wv_d.ap().rearrange("(o p) m -> o p m", p=128)
    Wpv = Wp_d.ap().rearrange("(o p) m -> o p m", p=128)
    W1v = W1_d.ap().rearrange("(o p) m -> o p m", p=128)
    W2v = W2_d.ap().rearrange("(o p) m -> p o m", p=128)  # [128, 32, 1024]
    xf = xf_d.ap()
    xo = xo_d.ap()
    xb = xbp_d.ap()

    for _rep in range(reps):
        _build_one(nc, locals())
    return IN_NAMES


def _build_one(nc, env):
    (xf_d, xo_d, xbp_d, qpos_d, Wq_d, Wk_d, Wv_d, Wp_d, W1_d, b1_d,
     W2_d, b2_d, qb_d, kb_d, vb_d, out_d, Wqv, Wkv, Wvv, Wpv, W1v, W2v,
     xf, xo, xb) = (
        env[k] for k in ["xf_d", "xo_d", "xbp_d", "qpos_d", "Wq_d",
                         "Wk_d", "Wv_d", "Wp_d", "W1_d", "b1_d", "W2_d",
                         "b2_d", "qb_d", "kb_d", "vb_d", "out_d",
                         "Wqv", "Wkv", "Wvv", "Wpv", "W1v", "W2v",
                         "xf", "xo", "xb"])
    import concourse.tile as tile
    from contextlib import ExitStack
    with tile.TileContext(nc) as tc, ExitStack() as top:
        const = top.enter_context(tc.tile_pool(name="const", bufs=1))
        eps_t = const.tile([128, 1], F32)
        nc.vector.memset(eps_t[:], EPS)

        def ln_block(pool, x4, h4, S):
            """Batched pre-LN: x4 [128, S, C] bf16 -> h4 [128, S, C] bf16.
            Phased so each engine gets runs of same-type ops (no ping-pong)."""
            n = C // 512
            stats = pool.tile([128, S, n, 6], F32, tag="ln_stats")
            mv = pool.tile([128, S, 2], F32, tag="ln_mv")
            for s in range(S):
                xg = x4[:, s].rearrange("p (n f) -> p n f", f=512)
                for i in range(n):
                    nc.vector.bn_stats(stats[:, s, i], xg[:, i])
                nc.vector.bn_aggr(mv[:, s], stats[:, s])
            rstd = pool.tile([128, S], F32, tag="ln_rstd")
            nc.scalar.activation(rstd[:], mv[:, :, 1], AF.Sqrt, bias=eps_t[:])
            nc.vector.reciprocal(rstd[:], rstd[:])
            nmr = pool.tile([128, S], F32, tag="ln_nmr")
            nc.vector.tensor_scalar(nmr[:], mv[:, :, 0], -1.0, None, op0=ALU.mult)
            nc.vector.tensor_tensor(nmr[:], nmr[:], rstd[:], ALU.mult)
            for s in range(S):
                nc.scalar.activation(h4[:, s], x4[:, s], AF.Identity,
                                     bias=nmr[:, s:s + 1], scale=rstd[:, s:s + 1])

        # persistent SBUF: x2 (residual after attn), token-major, bf16
        x2p = top.enter_context(tc.tile_pool(name="x2sb", bufs=1))
        x2_sb = x2p.tile([128, NB, C], BF16)

        # persistent attention tensors (entered early for pool-stack order)
        es_qkv = ExitStack()
        vp = es_qkv.enter_context(tc.tile_pool(name="Vp", bufs=1))
        V_sb = vp.tile([128, KB, H, 65], BF16)
        ktp = es_qkv.enter_context(tc.tile_pool(name="KTp", bufs=1))
        KT = ktp.tile([128, CCH, T], BF16)
        qtp = es_qkv.enter_context(tc.tile_pool(name="QTp", bufs=1))
        QT = qtp.tile([128, CCH, TOK], BF16)

        _mark(nc, 'A2_ln1own')
        # ==== Stage A'+B3 fused: LN1 of own rows -> Q projection, emitted
        # FIRST so the Q matmuls fill the PE while stage A's pipeline warms ====
        es_wq = ExitStack()
        wqp = es_wq.enter_context(tc.tile_pool(name="wqp", bufs=1))
        wq_a = wqp.tile([128, CCH, C], BF16)
        qb_sb = wqp.tile([128, NB], F32)
        with tc.tile_pool(name="stA2", bufs=2) as stA2, \
             tc.tile_pool(name="stA2x", bufs=2) as stA2x, \
             tc.tile_pool(name="stB3_ps", bufs=3, space="PSUM") as psB3:
            for tb4 in range(NB // 4):
                x4 = stA2x.tile([128, 4, C], BF16, tag="x4b")
                nc.scalar.dma_start(
                    x4[:], xo[tb4 * 512:(tb4 + 1) * 512, :]
                    .rearrange("(s p) c -> p s c", p=128))
                if tb4 == 0:
                    nc.scalar.dma_start(wq_a[:], Wqv.transpose([1, 0, 2]))
                    nc.scalar.dma_start(qb_sb[:], qb_d.ap().rearrange("o p -> p o"))
                h4 = stA2.tile([128, 4, C], BF16, tag="h4b")
                ln_block(stA2, x4, h4, 4)
                hTb = stA2.tile([128, CCH, 512], BF16, tag="hTb")
                for s in range(4):
                    nc.sync.dma_start(hTb[:, :, s * 128:(s + 1) * 128], h4[:, s],
                                      transpose=True)
                for pair in range(CCH):
                    pq = psB3.tile([128, 512], F32, tag="pq")
                    for cc in range(CCH):
                        nc.tensor.matmul(pq[:], wq_a[:, cc, pair * 128:(pair + 1) * 128],
                                         hTb[:, cc, :],
                                         start=(cc == 0), stop=(cc == CCH - 1))
                    nc.vector.tensor_scalar(
                        QT[:, pair, tb4 * 512:(tb4 + 1) * 512], pq[:],
                        qb_sb[:, pair:pair + 1], None, op0=ALU.add)
        es_wq.close()  # free Q weights

        _mark(nc, 'A_ln1full')
        # ====== Stage A: LN1 over full T -> hT [128, CCH, T] bf16 (XBAR) ======
        es_h = ExitStack()
        hp = es_h.enter_context(tc.tile_pool(name="hT", bufs=1, side="right"))
        hT = hp.tile([128, CCH, T], BF16)
        es_wv = ExitStack()
        wvp = es_wv.enter_context(tc.tile_pool(name="wvp", bufs=1, side="right"))
        wv_a = wvp.tile([128, CCH, C], BF16)
        vb_b = wvp.tile([128, C], F32)
        with tc.tile_pool(name="stA", bufs=2) as stA, \
             tc.tile_pool(name="stAx", bufs=2) as stAx:
            for tb4 in range(T // 512):
                x4 = stAx.tile([128, 4, C], BF16, tag="x4")
                nc.scalar.dma_start(
                    x4[:], xf[tb4 * 512:(tb4 + 1) * 512, :]
                    .rearrange("(s p) c -> p s c", p=128))
                if tb4 == 0:
                    # prefetch B1's weights right behind the first x batch
                    nc.scalar.dma_start(wv_a[:], Wvv.transpose([1, 0, 2]))
                    nc.scalar.dma_start(vb_b[:], vb_d.ap().to_broadcast([128, C]))
                h4 = stA.tile([128, 4, C], BF16, tag="h4")
                ln_block(stA, x4, h4, 4)
                for s in range(4):
                    tb = tb4 * 4 + s
                    nc.sync.dma_start(hT[:, :, tb * 128:(tb + 1) * 128], h4[:, s],
                                      transpose=True)

        _mark(nc, 'B1_V')
        # ============ Stage B1: V (token-major, bf16, ones-augmented) ============
        ones_f = vp.tile([128, 1], F32)
        nc.vector.memset(ones_f[:], 1.0)
        ones_r = vp.tile([128, 1], BF16)
        nc.vector.tensor_copy(ones_r[:], ones_f[:])
        nc.vector.tensor_copy(V_sb[:, :, :, 64:65],
                              ones_r[:, 0:1, None, None].to_broadcast([128, KB, H, 1]))
        with tc.tile_pool(name="stB1_ps", bufs=3, space="PSUM") as psB1:
            for grp in range(2):
                for tb in range(KB):
                    pv = psB1.tile([128, 512], F32, tag="pv")
                    for cc in range(CCH):
                        nc.tensor.matmul(pv[:], hT[:, cc, tb * 128:(tb + 1) * 128],
                                         wv_a[:, cc, grp * 512:(grp + 1) * 512],
                                         start=(cc == 0), stop=(cc == CCH - 1))
                    nc.vector.tensor_tensor(
                        V_sb[:, tb, grp * 8:(grp + 1) * 8, 0:64],
                        pv[:].rearrange("p (h d) -> p h d", d=64),
                        vb_b[:, grp * 512:(grp + 1) * 512].rearrange("p (h d) -> p h d", d=64),
                        ALU.add)
        es_wv.close()  # free V weights

        _mark(nc, 'B2_K')
        # ============ Stage B2: KT [128(dh pair-stacked), pair, T] bf16 ============
        with tc.tile_pool(name="stB2c", bufs=1) as stB2c, \
             tc.tile_pool(name="stB2_ps", bufs=3, space="PSUM") as psB2:
            kb_sb = stB2c.tile([128, NB], F32)
            nc.scalar.dma_start(kb_sb[:], kb_d.ap().rearrange("o p -> p o"))
            wk_a = stB2c.tile([128, CCH, C], BF16)
            nc.scalar.dma_start(wk_a[:], Wkv.transpose([1, 0, 2]))
            for pair in range(CCH):
                for nt in range(T // 512):
                    pk = psB2.tile([128, 512], F32, tag="pk")
                    for cc in range(CCH):
                        nc.tensor.matmul(pk[:], wk_a[:, cc, pair * 128:(pair + 1) * 128],
                                         hT[:, cc, nt * 512:(nt + 1) * 512],
                                         start=(cc == 0), stop=(cc == CCH - 1))
                    nc.vector.tensor_scalar(KT[:, pair, nt * 512:(nt + 1) * 512], pk[:],
                                            kb_sb[:, pair:pair + 1], None, op0=ALU.add)
        es_h.close()  # free hT (after B1+B2 consumed it)


        # attn output, dh-major in SBUF: [128(pair-part), CCH, TOK] bf16
        es_attn = ExitStack()
        atp = es_attn.enter_context(tc.tile_pool(name="attnT", bufs=1, side="right"))
        attnT_sb = atp.tile([128, CCH, TOK], BF16)

        # ---------- mask constants (scoped to attention) ----------
        es_mask = ExitStack()
        maskp = es_mask.enter_context(tc.tile_pool(name="maskp", bufs=1, side="right"))
        kp_i = maskp.tile([128, KB], mybir.dt.int32)
        nc.gpsimd.iota(kp_i[:], pattern=[[128, KB]], base=0, channel_multiplier=1)
        kp_f = maskp.tile([128, KB], F32)
        nc.vector.tensor_copy(kp_f[:], kp_i[:])
        qb = maskp.tile([128, NB, 128], F32)
        nc.scalar.dma_start(qb[:], qpos_d.ap()[None].to_broadcast([128, NB, 128]))
        biasm = maskp.tile([128, NB, 2, 128], BF16)
        for j in range(NB):
            for t in range(2):
                # m01[p_key, f_q] = (qpos_j[f] >= keypos(k=2j+t)[p])
                nc.vector.tensor_scalar(
                    biasm[:, j, t], qb[:, j], kp_f[:, 2 * j + t:2 * j + t + 1], None,
                    op0=ALU.is_ge)

        _mark(nc, 'C_attn')
        # ============ Stage C: attention ============
        with tc.tile_pool(name="stC", bufs=4) as stC, \
             tc.tile_pool(name="stCn", bufs=2) as stCn, \
             tc.tile_pool(name="stC_raw", bufs=1) as stCr, \
             tc.tile_pool(name="stC_att_ps", bufs=2, space="PSUM") as psCa, \
             tc.tile_pool(name="stC_s_ps", bufs=2, space="PSUM") as psCs:
            att_raw = stCr.tile([128, H, TOK], BF16)
            for h in range(H):
                pair, off = h // 2, 64 * (h % 2)
                ps_att = psCa.tile([128, TOK], F32, tag="ps_att")
                for k in range(KB):
                    jmin = k // 2
                    q0 = jmin * 128
                    nq = TOK - q0
                    weiT = stC.tile([128, TOK], BF16, tag="weiT")
                    ps_s = psCs.tile([128, 2, 512], F32, tag="ps_s")
                    qa = 0
                    while qa < nq:  # one matmul per 512-col psum bank
                        qn = min(512, nq - qa)
                        nc.tensor.matmul(
                            ps_s[:, qa // 512, 0:qn],
                            KT[off:off + 64, pair, k * 128:(k + 1) * 128],
                            QT[off:off + 64, pair, q0 + qa:q0 + qa + qn],
                            start=True, stop=True)
                        qa += qn
                    # one exp covering both banks
                    nc.scalar.activation(
                        weiT[:, 0:nq],
                        ps_s[:].rearrange("p a b -> p (a b)")[:, 0:nq],
                        AF.Exp, scale=0.125)
                    nc.vector.tensor_tensor(weiT[:, 0:128], weiT[:, 0:128],
                                            biasm[:, jmin, k - 2 * jmin], ALU.mult)
                    # AV: one matmul per 512-col PSUM bank (start=True must
                    # clear a whole bank, so groups are bank-aligned)
                    if k <= 7:  # bank 0: q cols [q0, 512)
                        nc.tensor.matmul(
                            ps_att[0:65, q0:512],
                            V_sb[:, k, h, :],
                            weiT[:, 0:512 - q0],
                            start=(k == 0), stop=(k == 7))
                    b1lo = max(512, q0)  # bank 1: q cols [b1lo, 1024)
                    nc.tensor.matmul(
                        ps_att[0:65, b1lo:TOK],
                        V_sb[:, k, h, :],
                        weiT[:, b1lo - q0:TOK - q0],
                        start=(k == 0), stop=(k == KB - 1))
                nc.vector.tensor_copy(att_raw[0:65, h], ps_att[0:65, :])
            # normalize: one XBAR transpose per 4 heads into q-major, then
            # pack head pairs into [q, j, ch] and transpose back once per pair
            for hg in range(H // 4):
                aq = stCn.tile([128, 4, NB, 128], BF16, tag="aq")
                nc.sync.dma_start(aq[:], att_raw[:, 4 * hg:4 * (hg + 1), :],
                                  transpose=True)
                recs = stCn.tile([128, 4, NB], F32, tag="recs")
                nc.vector.reciprocal(recs[:], aq[:, :, :, 64])
                for tp in range(2):
                    pair = 2 * hg + tp
                    ap_pair = stCn.tile([128, NB, 128], BF16, tag="ap_pair")
                    for t in range(2):
                        hh = 2 * tp + t   # head index within this 4-head group
                        for j in range(NB):
                            nc.vector.tensor_scalar_mul(
                                ap_pair[:, j, t * 64:(t + 1) * 64],
                                aq[:, hh, j, 0:64], recs[:, hh, j:j + 1])
                    nc.sync.dma_start(
                        attnT_sb[:, pair, :].rearrange("p (j f) -> p j f", f=128),
                        ap_pair[:], transpose=True)
        es_mask.close()  # free mask constants
        es_qkv.close()   # free V, KT, QT

        _mark(nc, 'D_wp_ln2')
        # ==== Stage D: Wp proj (token-major out) + residual(+bp) + LN2 ====
        es_h2 = ExitStack()
        h2p = es_h2.enter_context(tc.tile_pool(name="h2T", bufs=1))
        h2T = h2p.tile([128, CCH, TOK], BF16)
        with tc.tile_pool(name="stD", bufs=2) as stD, \
             tc.tile_pool(name="stDc", bufs=1) as stDc, \
             tc.tile_pool(name="stD_ps", bufs=2, space="PSUM") as psD:
            wp_a = stDc.tile([128, CCH, C], BF16)
            nc.scalar.dma_start(wp_a[:], Wpv.transpose([1, 0, 2]))
            for nt in range(TOK // 512):
                xbp4 = stD.tile([128, 4, C], F32, tag="xbp4")
                nc.scalar.dma_start(
                    xbp4[:], xb[nt * 512:(nt + 1) * 512, :]
                    .rearrange("(s p) c -> p s c", p=128))
                for sub in range(4):
                    tb = nt * 4 + sub
                    # pp[tok, C] = attn[tok, :] @ Wp  (attnT chunks stationary)
                    pp = psD.tile([128, C], F32, tag="pp")
                    for half in range(2):
                        for cc in range(CCH):
                            nc.tensor.matmul(
                                pp[:, half * 512:(half + 1) * 512],
                                attnT_sb[:, cc, tb * 128:(tb + 1) * 128],
                                wp_a[:, cc, half * 512:(half + 1) * 512],
                                start=(cc == 0), stop=(cc == CCH - 1))
                    nc.vector.tensor_tensor(x2_sb[:, tb], pp[:], xbp4[:, sub],
                                            ALU.add)
                # LN2 (batched over the 4 token blocks)
                h2r4 = stD.tile([128, 4, C], BF16, tag="h2r4")
                ln_block(stD, x2_sb[:, nt * 4:(nt + 1) * 4], h2r4, 4)
                for sub in range(4):
                    tb = nt * 4 + sub
                    nc.sync.dma_start(h2T[:, :, tb * 128:(tb + 1) * 128],
                                      h2r4[:, sub], transpose=True)
        es_attn.close()  # free attnT

        _mark(nc, 'X_b2fold')
        # fold b2 into x2 now that LN2 has consumed x2 (out = x2 + b2 + ff);
        # also prefetch the full W2 for the token-major second MLP matmul
        es_w2 = ExitStack()
        w2p = es_w2.enter_context(tc.tile_pool(name="w2p", bufs=1))
        w2_a = w2p.tile([128, FCH, C], BF16)
        for g in range(4):
            nc.scalar.dma_start(w2_a[:, g * 8:(g + 1) * 8], W2v[:, g * 8:(g + 1) * 8, :])
        with tc.tile_pool(name="stX2b", bufs=1) as stX2b:
            b2b = stX2b.tile([128, C], F32)
            nc.scalar.dma_start(b2b[:], b2_d.ap().to_broadcast([128, C]))
            for tb in range(NB):
                nc.vector.tensor_tensor(x2_sb[:, tb], x2_sb[:, tb], b2b[:], ALU.add)

        _mark(nc, 'E1_w1')
        # ============ Stage E: single-pass MLP (ff1T bf16 fits SBUF) ============
        es_ff = ExitStack()
        ffp = es_ff.enter_context(tc.tile_pool(name="ff1T", bufs=1))
        ff1T = ffp.tile([128, FCH, TOK], BF16)
        with tc.tile_pool(name="stE_c", bufs=1) as stEc:
            b1p = stEc.tile([128, FCH], F32)
            nc.scalar.dma_start(b1p[:], b1_d.ap().rearrange("x (o p) -> p (x o)", p=128))
            with tc.tile_pool(name="stE1", bufs=2) as stE1, \
                 tc.tile_pool(name="stE1_ps", bufs=3, space="PSUM") as psE1:
                for fog in range(4):
                    w1g = stE1.tile([128, CCH, 1024], BF16, tag="w1g")
                    nc.scalar.dma_start(
                        w1g[:], W1v.transpose([1, 0, 2])[:, :, fog * 1024:(fog + 1) * 1024])
                    for f4 in range(8):
                        fo = fog * 8 + f4
                        for nt in range(TOK // 512):
                            pf = psE1.tile([128, 512], F32, tag="pf")
                            for cc in range(CCH):
                                nc.tensor.matmul(
                                    pf[:], w1g[:, cc, f4 * 128:(f4 + 1) * 128],
                                    h2T[:, cc, nt * 512:(nt + 1) * 512],
                                    start=(cc == 0), stop=(cc == CCH - 1))
                            nc.scalar.activation(
                                ff1T[:, fo, nt * 512:(nt + 1) * 512], pf[:], AF.Relu,
                                bias=b1p[:, fo:fo + 1])
            _mark(nc, 'E2F_w2out')
            # ====== Stage E2+F: W2 (token-major out), +x2(+b2), store ======
            with tc.tile_pool(name="stE2", bufs=2) as stE2, \
                 tc.tile_pool(name="stE2_ps", bufs=2, space="PSUM") as psE2:
                for tb in range(NB):
                    p2 = psE2.tile([128, C], F32, tag="p2")
                    for half in range(2):
                        for fo in range(FCH):
                            nc.tensor.matmul(
                                p2[:, half * 512:(half + 1) * 512],
                                ff1T[:, fo, tb * 128:(tb + 1) * 128],
                                w2_a[:, fo, half * 512:(half + 1) * 512],
                                start=(fo == 0), stop=(fo == FCH - 1))
                    out_t = stE2.tile([128, C], F32, tag="out_t")
                    nc.vector.tensor_tensor(out_t[:], p2[:], x2_sb[:, tb], ALU.add)
                    nc.sync.dma_start(out_d.ap()[tb * 128:(tb + 1) * 128, :], out_t[:])
        es_ff.close()
        es_w2.close()
        es_h2.close()  # free h2T (kept open past E1 for pool stack order)


def make_nc():
    nc = bacc.Bacc("TRN2", target_bir_lowering=False, debug=False,
                   num_devices=N_CORES)
    build(nc)
    nc.compile()
    return nc


def shard_inputs(inputs):
    """Full inputs dict -> list of 8 per-core in_maps.

    Folds LN1 gain/bias into Wq/Wk/Wv (weights scaled by g1 per input channel,
    be1 contribution becomes an additive bias on q/k/v) and LN2's into W1/b1.
    Weight matrices and the LN inputs are cast to bf16; the residual stream
    (xbp = xown + bp) stays f32.
    """
    import ml_dtypes
    bf16 = ml_dtypes.bfloat16
    x = np.asarray(inputs["x"], np.float32)
    assert x.shape == (B, T, C)
    f64 = np.float64
    Wq = np.asarray(inputs["Wq"], f64); Wk = np.asarray(inputs["Wk"], f64)
    Wv = np.asarray(inputs["Wv"], f64); Wp = np.asarray(inputs["Wp"], f64)
    W1 = np.asarray(inputs["W1"], f64); W2 = np.asarray(inputs["W2"], f64)
    g1 = np.asarray(inputs["g1"], f64); be1 = np.asarray(inputs["be1"], f64)
    g2 = np.asarray(inputs["g2"], f64); be2 = np.asarray(inputs["be2"], f64)
    b1 = np.asarray(inputs["b1"], f64)
    bp = np.asarray(inputs["bp"], np.float32)
    shared = {
        "Wq": (g1[:, None] * Wq).astype(bf16),
        "Wk": (g1[:, None] * Wk).astype(bf16),
        "Wv": (g1[:, None] * Wv).astype(bf16),
        "Wp": Wp.astype(bf16), "W2": W2.astype(bf16),
        "W1": (g2[:, None] * W1).astype(bf16),
        "qbias": (be1 @ Wq).astype(np.float32).reshape(NB, 128),
        "kbias": (be1 @ Wk).astype(np.float32).reshape(NB, 128),
        "vbias": (be1 @ Wv).astype(np.float32).reshape(1, C),
        "b1": (b1 + be2 @ W1).astype(np.float32).reshape(1, FF),
        "b2": np.asarray(inputs["b2"], np.float32).reshape(1, C),
    }
    in_maps = []
    for c in range(N_CORES):
        b, par = c // 2, c % 2
        gblocks = [2 * j + par for j in range(NB)]
        rows = np.concatenate([x[b, g * 128:(g + 1) * 128, :] for g in gblocks], 0)
        qpos = np.stack([np.arange(g * 128, (g + 1) * 128, dtype=np.float32)
                         for g in gblocks], 0)
        m = {"xfbf": np.ascontiguousarray(x[b]).astype(bf16),
             "xobf": np.ascontiguousarray(rows).astype(bf16),
             "xbp": np.ascontiguousarray(rows + bp[None, :]),
             "qpos": qpos}
        m.update(shared)
        in_maps.append(m)
    return in_maps


def unshard_outputs(results):
    """list of per-core {'out': [TOK, C]} -> [B, T, C]"""
    out = np.zeros((B, T, C), np.float32)
    for c in range(N_CORES):
        b, par = c // 2, c % 2
        r = np.asarray(results[c]["out"])
        for j in range(NB):
            g = 2 * j + par
            out[b, g * 128:(g + 1) * 128, :] = r[j * 128:(j + 1) * 128, :]
    return out


_NC_CACHE = {}

def _get_nc():
    if "nc" not in _NC_CACHE:
        nc = bacc.Bacc("TRN2", target_bir_lowering=False, debug=False,
                       num_devices=N_CORES)
        build(nc, reps=1)
        nc.compile()
        _NC_CACHE["nc"] = nc
    return _NC_CACHE["nc"]


def kernel(**inputs):
    from concourse.bass_utils import run_bass_kernel_spmd
    nc = _get_nc()
    in_maps = shard_inputs(inputs)
    res = run_bass_kernel_spmd(nc, in_maps, core_ids=list(range(N_CORES)))
    return unshard_outputs(res.results)
